# revision 1
# baseline (speedup 1.0000x reference)
"""Trainium2 Bass kernel for MHSA with Transformer-XL relative position bias.

Problem: B=16, T=1024, DM=256, H=4, HS=64 fp32.
Sharding: pure data-parallel over batch across 8 cores (2 batches/core).

Per-core pipeline (M = 2*1024 = 2048 rows):
  1. LN in [m, d] layout (bn_stats), PE-transpose -> xnT/posT [256, M] bf16
  2. Projections via PE: QuT/QvT/KT/PT [256, M] bf16 (s on partitions), V [M, 256] bf16
  3. Per (b, h): pos scores X = QvT.T @ PT -> PSUM -> bf16 -> DRAM scratch
     [1024, 1025] (col 0 zeroed); rel_shift = re-read with row-stride 1024 from
     element offset 1024 (Transformer-XL shift == flat-buffer shear);
     content scores C = QuT.T @ KT into PSUM, R added into the same PSUM via
     identity matmul; exp((C+R)/8) on ACT with fused row-sum (logits are small:
     max |logit| ~ 1.2, so no max subtraction); A = E * (1/S) in bf16;
     A transposed n<->m via xbar DMA (SBUF->SBUF); AV^T accumulated on PE.
  4. Out-proj from AVT (f32r), + bo + residual, DMA out.
"""
import sys

sys.path.insert(0, "/opt/trn_rl_repo")

import numpy as np

import concourse.bass as bass
import concourse.bacc as bacc
import concourse.tile as tile
from concourse import mybir
from concourse.masks import make_identity
from concourse.bass_utils import run_bass_kernel_spmd

B, T, DM, H, HS = 16, 1024, 256, 4, 64
NCORES = 8
BL = B // NCORES          # local batches per core
M = BL * T                # local rows (2048)
NMT = M // 128            # m-tiles (16)
P = 128
LN_EPS = 1e-3
F32 = mybir.dt.float32
F32R = mybir.dt.float32r
BF16 = mybir.dt.bfloat16


def build_bass():
    nc = bacc.Bacc("TRN2", target_bir_lowering=False, debug=False,
                   enable_asserts=False, num_devices=NCORES)

    x_in = nc.dram_tensor("x", [M, DM], F32, kind="ExternalInput").ap()
    pos_in = nc.dram_tensor("pos", [M, DM], F32, kind="ExternalInput").ap()
    wq_in = nc.dram_tensor("wq", [DM, DM], F32, kind="ExternalInput").ap()
    wk_in = nc.dram_tensor("wk", [DM, DM], F32, kind="ExternalInput").ap()
    wv_in = nc.dram_tensor("wv", [DM, DM], F32, kind="ExternalInput").ap()
    wp_in = nc.dram_tensor("wp", [DM, DM], F32, kind="ExternalInput").ap()
    wo_in = nc.dram_tensor("wo", [DM, DM], F32, kind="ExternalInput").ap()
    bqu_in = nc.dram_tensor("bqu", [DM], F32, kind="ExternalInput").ap()
    bqv_in = nc.dram_tensor("bqv", [DM], F32, kind="ExternalInput").ap()
    bk_in = nc.dram_tensor("bk", [DM], F32, kind="ExternalInput").ap()
    bo_in = nc.dram_tensor("bo", [DM], F32, kind="ExternalInput").ap()
    out = nc.dram_tensor("out", [M, DM], F32, kind="ExternalOutput").ap()

    scr = [
        nc.dram_tensor(f"xscr{i}", [T, T + 1], BF16, kind="Internal").ap()
        for i in range(2)
    ]

    with tile.TileContext(nc) as tc:
        with tc.tile_pool(name="persist", bufs=1) as pp:
            # --- persistent SBUF ---
            ident = pp.tile([P, P], F32)
            make_identity(nc, ident)
            ident_bf = pp.tile([P, P], BF16)
            nc.gpsimd.tensor_copy(out=ident_bf, in_=ident)

            def load_w(ap_in, dtype, name):
                ts = [pp.tile([P, DM], dtype, tag=f"{name}{c}", name=f"{name}{c}") for c in range(2)]
                for c in range(2):
                    if dtype == F32:
                        nc.sync.dma_start(out=ts[c], in_=ap_in[c * P:(c + 1) * P, :])
                    else:
                        tmp = pp.tile([P, DM], F32, tag=f"{name}tmp{c}", name=f"{name}tmp{c}")
                        nc.sync.dma_start(out=tmp, in_=ap_in[c * P:(c + 1) * P, :])
                        nc.gpsimd.tensor_copy(out=ts[c], in_=tmp)
                return ts

            wq_sb = load_w(wq_in, BF16, "wq")
            wk_sb = load_w(wk_in, BF16, "wk")
            wv_sb = load_w(wv_in, BF16, "wv")
            wp_sb = load_w(wp_in, BF16, "wp")
            wo_sb = load_w(wo_in, BF16, "wo")

            def load_col(ap_in, name):
                ts = [pp.tile([P, 1], F32, tag=f"{name}{c}", name=f"{name}{c}") for c in range(2)]
                for c in range(2):
                    nc.sync.dma_start(
                        out=ts[c],
                        in_=bass.AP(tensor=ap_in.tensor, offset=c * P, ap=[[1, P], [1, 1]]),
                    )
                return ts

            bqu_c = load_col(bqu_in, "bqu")
            bqv_c = load_col(bqv_in, "bqv")
            bk_c = load_col(bk_in, "bk")

            def load_bcast(ap_in, name):
                t = pp.tile([P, DM], F32, tag=f"{name}b", name=f"{name}b")
                nc.sync.dma_start(
                    out=t,
                    in_=bass.AP(tensor=ap_in.tensor, offset=0, ap=[[0, P], [1, DM]]),
                )
                return t

            bo_b = load_bcast(bo_in, "bo")

            eps_t = pp.tile([P, 1], F32)
            nc.vector.memset(eps_t, LN_EPS)

            x_res = pp.tile([P, NMT, DM], F32)        # residual copy of inputs
            xnT = [pp.tile([P, M], BF16, tag=f"xnT{c}", name=f"xnT{c}") for c in range(2)]
            posT = [pp.tile([P, M], BF16, tag=f"posT{c}", name=f"posT{c}") for c in range(2)]
            quT = [pp.tile([P, M], BF16, tag=f"quT{c}", name=f"quT{c}") for c in range(2)]
            qvT = [pp.tile([P, M], BF16, tag=f"qvT{c}", name=f"qvT{c}") for c in range(2)]
            kT = [pp.tile([P, M], BF16, tag=f"kT{c}", name=f"kT{c}") for c in range(2)]
            pT = [pp.tile([P, M], BF16, tag=f"pT{c}", name=f"pT{c}") for c in range(2)]
            v_sb = pp.tile([P, NMT, DM], BF16)        # V[mt*128+p, s] at [:, mt, s]
            avT = [pp.tile([P, M], BF16, tag=f"avT{c}", name=f"avT{c}") for c in range(2)]

            # ---------------- phase 1: LN + transposes ----------------
            with tc.tile_pool(name="ph1", bufs=3) as sb1, \
                 tc.tile_pool(name="ps1", bufs=4, space="PSUM") as ps1:
                for mt in range(NMT):
                    xs = x_res[:, mt, :]
                    nc.sync.dma_start(out=xs, in_=x_in[mt * P:(mt + 1) * P, :])
                    stats = sb1.tile([P, 6], F32, tag="stats")
                    nc.vector.bn_stats(out=stats, in_=xs)
                    mv = sb1.tile([P, 2], F32, tag="mv")
                    nc.vector.bn_aggr(out=mv, in_=stats)
                    rstd = sb1.tile([P, 1], F32, tag="rstd")
                    nc.scalar.activation(out=rstd, in_=mv[:, 1:2],
                                         func=mybir.ActivationFunctionType.Sqrt,
                                         bias=eps_t, scale=1.0)
                    nc.vector.reciprocal(out=rstd, in_=rstd)
                    xn = sb1.tile([P, DM], F32, tag="xn")
                    nc.vector.tensor_scalar(out=xn, in0=xs, scalar1=mv[:, 0:1],
                                            scalar2=rstd,
                                            op0=mybir.AluOpType.subtract,
                                            op1=mybir.AluOpType.mult)
                    pt = sb1.tile([P, DM], F32, tag="pt")
                    nc.sync.dma_start(out=pt, in_=pos_in[mt * P:(mt + 1) * P, :])
                    for c in range(2):
                        tp = ps1.tile([P, P], F32, tag="tp")
                        nc.tensor.transpose(tp, xn[:, c * P:(c + 1) * P], ident)
                        nc.scalar.copy(out=xnT[c][:, mt * P:(mt + 1) * P], in_=tp)
                        tp2 = ps1.tile([P, P], F32, tag="tp")
                        nc.tensor.transpose(tp2, pt[:, c * P:(c + 1) * P], ident)
                        nc.scalar.copy(out=posT[c][:, mt * P:(mt + 1) * P], in_=tp2)

            # ---------------- phase 2: projections ----------------
            with tc.tile_pool(name="ps2", bufs=2, space="PSUM") as ps2:
                for sc in range(2):
                    for mc in range(4):
                        msl = slice(mc * 512, (mc + 1) * 512)
                        pq = ps2.tile([P, 512], F32, tag="pq")
                        pk = ps2.tile([P, 512], F32, tag="pk")
                        pps = ps2.tile([P, 512], F32, tag="pp")
                        for dc in range(2):
                            nc.tensor.matmul(pq, lhsT=wq_sb[dc][:, sc * P:(sc + 1) * P],
                                             rhs=xnT[dc][:, msl],
                                             start=(dc == 0), stop=(dc == 1))
                            nc.tensor.matmul(pk, lhsT=wk_sb[dc][:, sc * P:(sc + 1) * P],
                                             rhs=xnT[dc][:, msl],
                                             start=(dc == 0), stop=(dc == 1))
                            nc.tensor.matmul(pps, lhsT=wp_sb[dc][:, sc * P:(sc + 1) * P],
                                             rhs=posT[dc][:, msl],
                                             start=(dc == 0), stop=(dc == 1))
                        nc.scalar.activation(out=quT[sc][:, msl], in_=pq,
                                             func=mybir.ActivationFunctionType.Identity,
                                             bias=bqu_c[sc], scale=1.0)
                        nc.scalar.activation(out=qvT[sc][:, msl], in_=pq,
                                             func=mybir.ActivationFunctionType.Identity,
                                             bias=bqv_c[sc], scale=1.0)
                        nc.vector.tensor_scalar_add(out=kT[sc][:, msl], in0=pk,
                                                    scalar1=bk_c[sc])
                        nc.vector.tensor_copy(out=pT[sc][:, msl], in_=pps)
                for mt in range(NMT):
                    pv = ps2.tile([P, DM], F32, tag="pv")
                    for dc in range(2):
                        nc.tensor.matmul(pv, lhsT=xnT[dc][:, mt * P:(mt + 1) * P],
                                         rhs=wv_sb[dc],
                                         start=(dc == 0), stop=(dc == 1))
                    nc.vector.tensor_copy(out=v_sb[:, mt, :], in_=pv)

            # ---------------- phase 3: attention per (b, h) ----------------
            with tc.tile_pool(name="ph3", bufs=3) as sb3, \
                 tc.tile_pool(name="at", bufs=2) as atp, \
                 tc.tile_pool(name="ps3", bufs=2, space="PSUM") as ps3, \
                 tc.tile_pool(name="ps3x", bufs=2, space="PSUM") as ps3x, \
                 tc.tile_pool(name="ps3av", bufs=2, space="PSUM") as ps3av:
                NBH = BL * H
                at_tiles = {}

                def stage_a(bh, mt):
                    b, h = divmod(bh, H)
                    hh, po = h // 2, (h % 2) * 64
                    ssl = slice(po, po + 64)
                    sc_t = scr[bh % 2]
                    mg = slice(b * T + mt * P, b * T + (mt + 1) * P)
                    xbf = sb3.tile([P, T + 1], BF16, tag="xbf", name="xbf")
                    nc.gpsimd.memset(xbf[:, 0:1], 0.0)
                    for nck in range(2):
                        xp = ps3x.tile([P, 512], F32, tag="xp", name="xp")
                        nc.tensor.matmul(
                            xp, lhsT=qvT[hh][ssl, mg],
                            rhs=pT[hh][ssl, b * T + nck * 512:b * T + (nck + 1) * 512],
                            start=True, stop=True)
                        osl = xbf[:, 1 + nck * 512:1 + (nck + 1) * 512]
                        if nck == 0:
                            nc.vector.tensor_copy(out=osl, in_=xp)
                        else:
                            nc.scalar.copy(out=osl, in_=xp)
                    nc.gpsimd.dma_start(out=sc_t[mt * P:(mt + 1) * P, :], in_=xbf)

                def stage_bc(bh, mt):
                    b, h = divmod(bh, H)
                    hh, po = h // 2, (h % 2) * 64
                    ssl = slice(po, po + 64)
                    sc_t = scr[bh % 2]
                    at = at_tiles[bh]
                    mg = slice(b * T + mt * P, b * T + (mt + 1) * P)
                    rbf = sb3.tile([P, T], BF16, tag="rbf", name="rbf")
                    nc.sync.dma_start(
                        out=rbf,
                        in_=bass.AP(tensor=sc_t.tensor, offset=T + mt * P * T,
                                    ap=[[T, P], [1, T]]))
                    cp = ps3.tile([P, T], F32, tag="big", name="cp")
                    for nck in range(2):
                        nc.tensor.matmul(
                            cp[:, nck * 512:(nck + 1) * 512], lhsT=quT[hh][ssl, mg],
                            rhs=kT[hh][ssl, b * T + nck * 512:b * T + (nck + 1) * 512],
                            start=True, stop=True)
                    lbf = sb3.tile([P, T], BF16, tag="lbf", name="lbf")
                    nc.vector.scalar_tensor_tensor(
                        out=lbf, in0=cp, scalar=0.0, in1=rbf,
                        op0=mybir.AluOpType.bypass, op1=mybir.AluOpType.add)
                    ebf = sb3.tile([P, T], BF16, tag="ebf", name="ebf")
                    ssum = sb3.tile([P, 1], F32, tag="ssum", name="ssum")
                    nc.scalar.activation(out=ebf, in_=lbf,
                                         func=mybir.ActivationFunctionType.Exp,
                                         scale=0.125, accum_out=ssum)
                    nc.vector.reciprocal(out=ssum, in_=ssum)
                    abf = sb3.tile([P, T], BF16, tag="abf", name="abf")
                    nc.vector.tensor_scalar_mul(out=abf, in0=ebf, scalar1=ssum)
                    nc.sync.dma_start_transpose(
                        out=at[:, :, mt * P:(mt + 1) * P], in_=abf)

                def stage_d(bh, avps, nt):
                    b, h = divmod(bh, H)
                    at = at_tiles[bh]
                    for mc in range(2):
                        nc.tensor.matmul(
                            avps[mc],
                            lhsT=v_sb[:, b * (T // P) + nt, h * HS:(h + 1) * HS],
                            rhs=at[:, nt, mc * 512:(mc + 1) * 512],
                            start=(nt == 0), stop=(nt == T // P - 1))

                def stage_d_out(bh, avps):
                    b, h = divmod(bh, H)
                    hh, po = h // 2, (h % 2) * 64
                    for mc in range(2):
                        nc.scalar.copy(
                            out=avT[hh][po:po + 64,
                                        b * T + mc * 512:b * T + (mc + 1) * 512],
                            in_=avps[mc])
                    del at_tiles[bh]

                avps_cur = None
                for step in range(NBH + 2):
                    if step - 1 >= 0 and step - 1 < NBH:
                        at_tiles[step - 1] = atp.tile([P, T // P, T], BF16,
                                                      tag="at", name="at")
                    if step - 2 >= 0:
                        avps_cur = [ps3av.tile([64, 512], F32, tag="av",
                                               name=f"avp{mc}") for mc in range(2)]
                    for mt in range(T // P):
                        if step < NBH:
                            stage_a(step, mt)
                        if 0 <= step - 1 < NBH:
                            stage_bc(step - 1, mt)
                        if step - 2 >= 0:
                            stage_d(step - 2, avps_cur, mt)
                    if step - 2 >= 0:
                        stage_d_out(step - 2, avps_cur)

            # ---------------- phase 4: out-proj + residual ----------------
            with tc.tile_pool(name="ph4", bufs=3) as sb4, \
                 tc.tile_pool(name="ps4", bufs=2, space="PSUM") as ps4:
                for mt in range(NMT):
                    op = ps4.tile([P, DM], F32, tag="op")
                    for sc in range(2):
                        nc.tensor.matmul(op,
                                         lhsT=avT[sc][:, mt * P:(mt + 1) * P],
                                         rhs=wo_sb[sc],
                                         start=(sc == 0), stop=(sc == 1))
                    ot = sb4.tile([P, DM], F32, tag="ot")
                    nc.vector.scalar_tensor_tensor(out=ot, in0=op, scalar=0.0,
                                                   in1=x_res[:, mt, :],
                                                   op0=mybir.AluOpType.bypass,
                                                   op1=mybir.AluOpType.add)
                    nc.vector.tensor_tensor(out=ot, in0=ot, in1=bo_b,
                                            op=mybir.AluOpType.add)
                    nc.sync.dma_start(out=out[mt * P:(mt + 1) * P, :], in_=ot)
    nc.finalize()
    return nc


_NC = None


def make_in_maps(inputs):
    f = lambda a: np.ascontiguousarray(np.asarray(a, dtype=np.float32))
    x = f(inputs["inputs"]).reshape(B, T, DM)
    pos = f(inputs["pos_enc"]).reshape(B, T, DM)
    wq0 = f(inputs["Wq"]).reshape(DM, DM)
    wk0 = f(inputs["Wk"]).reshape(DM, DM)
    wv0 = f(inputs["Wv"]).reshape(DM, DM)
    wp = f(inputs["Wp"]).reshape(DM, DM)
    wo = f(inputs["Wo"]).reshape(DM, DM)
    gamma = f(inputs["gamma"]).reshape(DM, 1)
    beta = f(inputs["beta"]).reshape(DM)
    # fold LN's gamma into the x-side weights, beta into the projection biases,
    # and bv through softmax (rows sum to 1) into the output bias
    wq, wk, wv = gamma * wq0, gamma * wk0, gamma * wv0
    bqu = (f(inputs["bq"]).reshape(DM) + f(inputs["pos_bias_u"]).reshape(DM)
           + beta @ wq0)
    bqv = (f(inputs["bq"]).reshape(DM) + f(inputs["pos_bias_v"]).reshape(DM)
           + beta @ wq0)
    bk = f(inputs["bk"]).reshape(DM) + beta @ wk0
    bv_eff = f(inputs["bv"]).reshape(DM) + beta @ wv0
    bo = f(inputs["bo"]) + bv_eff @ wo
    shared = dict(
        wq=wq, wk=wk, wv=wv, wp=wp, wo=wo,
        bqu=bqu, bqv=bqv, bk=bk, bo=bo,
    )
    in_maps = []
    for c in range(NCORES):
        sl = slice(c * BL, (c + 1) * BL)
        in_maps.append(dict(
            x=np.ascontiguousarray(x[sl].reshape(M, DM)),
            pos=np.ascontiguousarray(pos[sl].reshape(M, DM)),
            **shared,
        ))
    return in_maps


def kernel(**inputs) -> np.ndarray:
    global _NC
    if _NC is None:
        _NC = build_bass()
    in_maps = make_in_maps(inputs)
    res = run_bass_kernel_spmd(_NC, in_maps, core_ids=list(range(NCORES)))
    outs = [r["out"].reshape(BL, T, DM) for r in res.results]
    return np.concatenate(outs, axis=0)

